# revision 23
# baseline (speedup 1.0000x reference)
"""Trainium2 Bass kernel for nn_CLCRNModel (CLCRN encoder-decoder GNN).

Strategy: data-parallel over batch (8 batch elements -> 8 NeuronCores).
The sparse 25-neighbor graph conv is cast as dense matmuls against the
row-normalized adjacency A and its square B = A^2, both SBUF-resident in
fp8-e4m3 and streamed through the PE with DoubleRow (2 fp8 MACs/cell).

Input-volume optimization: every core receives only a 2-k-tile slice of
A^T (512 KB fp8) plus a 1/8 slice of a packed weight blob; the full A is
assembled on device with an HBM AllGather, its natural-layout copy is
built with PE transposes, and B = A@A is computed on the PE (fp8 DR)
directly into the SBUF hop operand. No dense matrix crosses the
host-device link.

Per cell the PE runs two fused hop passes -- [A@z | B@z] accumulate into
disjoint halves of one PSUM bank per 512-chunk, so each k-tile-pair's
stationary is loaded once per pass (links-outer) and each chunk needs a
single PSUM->SBUF copy. Dense gate/cand matmuls are grouped as
[h|node|x] (bf16, K=115) and [A@z|B@z] (K=128) moving operands.

Host-side linear-algebra folds shrink every hop pass to the 64 hidden
channels: encoder feature-embedding and node embedding fold into
precomputed dense rows/biases (exact f32 A), and the decoder feedback
y_t = h_t @ W_proj + b_proj folds into the h-group dense weights.
"""
import os
import sys

for _p in ("/root/.axon_site/_ro/trn_rl_repo", "/opt/trn_rl_repo"):
    if os.path.isdir(_p) and _p not in sys.path:
        sys.path.append(_p)

import numpy as np
import ml_dtypes

import concourse.bass as bass
import concourse.mybir as mybir
import concourse.tile as tile
from concourse.bass_utils import run_bass_kernel_spmd
from concourse.masks import make_identity

P = 128
N = 2048
NT = 16            # node k-tiles
NPAIR = 8          # DoubleRow k-tile pairs
S = 12             # encoder steps
HOR = 12           # decoder steps
H = 64             # GRU units
FREE = 512         # hop chunk width (fp8 DR moving limit: 2x512)
NCH = N // FREE
NCORES = 8
KSL = NT // NCORES  # k-tiles per core slice
SA = 16.0          # fp8 scale for A
SB = 128.0         # fp8 scale for B (= SA^2 * 0.5, applied in B-build copy)

F32 = mybir.dt.float32
BF16 = mybir.dt.bfloat16
FP8 = mybir.dt.float8e4
AF = mybir.ActivationFunctionType
DR = mybir.MatmulPerfMode.DoubleRow

# packed weight blob (bf16-unit offsets). kind "bf16": [rows, cols] bf16
# tile; kind "fp8": [64, 2, cols] fp8 DoubleRow-paired tile, stored as raw
# byte pairs inside the bf16 blob (AP.bitcast on device).
# nodeT is [node; A@node; B@node].T
_WLAYOUT = [
    ("nodeT", 48, N, "bf16"),
    ("wge1", 115, 128, "bf16"), ("wge2", 64, 128, "fp8"),
    ("wce1", 115, 64, "bf16"), ("wce2", 64, 64, "fp8"),
    ("wgd01", 64, 128, "bf16"), ("wgd02", 64, 128, "fp8"),
    ("wgdf1", 64, 128, "bf16"), ("wgdf2", 64, 128, "fp8"),
    ("wcd01", 64, 64, "bf16"), ("wcd02", 64, 64, "fp8"),
    ("wcdy1", 64, 64, "bf16"), ("wcdy2", 64, 64, "fp8"),
    ("wproj", 64, 1, "bf16"),
]


def _wsize(r, c, kind):
    # bf16-unit count in the blob: fp8 parts are [r, 2, c] fp8 = r*c units
    return r * c if kind == "bf16" else r * c


WTOT = sum(_wsize(r, c, k) for _, r, c, k in _WLAYOUT)
assert WTOT % NCORES == 0
WSL = WTOT // NCORES


def _dedup_ldweights(nc):
    """Remove Ldweights whose weights AP equals the previous PE weight
    load (PE retains the stationary operand between matmuls; walrus's own
    ldw-opt is disabled in this toolchain). Waits/updates of a removed
    load migrate to the next PE instruction."""
    import concourse.mybir as _mb
    fn = nc.m.functions[0]
    pe = _mb.EngineType.PE
    n = 0
    for blk in fn.blocks:
        out = []
        last_sig = None
        pend_waits, pend_updates = [], []
        for ins in blk.instructions:
            if ins.engine == pe:
                if ins.opcode == "Ldweights":
                    sig = (str(ins.ins[0]), str(ins.tile_position),
                           str(ins.perf_mode), str(ins.is_transpose))
                    if sig == last_sig:
                        si = ins.sync_info
                        if si:
                            pend_waits.extend(si.on_wait or [])
                            pend_updates.extend(si.on_update or [])
                        n += 1
                        continue
                    last_sig = sig
                elif ins.opcode not in ("Matmult", "Drain", "EventSemaphore",
                                        "RegisterMove", "UnconditionalBranch"):
                    last_sig = None
                if pend_waits or pend_updates:
                    si = ins.sync_info
                    if si is None:
                        si = _mb.SyncInfo(on_wait=[], on_update=[])
                        ins.sync_info = si
                    si.on_wait = list(pend_waits) + list(si.on_wait or [])
                    si.on_update = list(si.on_update or []) + list(pend_updates)
                    pend_waits, pend_updates = [], []
            out.append(ins)
        assert not pend_waits and not pend_updates
        blk.instructions = out
    return n


def _split_multiwait(nc, max_waits=1):
    """This container's walrus rejects >1 sem-wait on CTRL-class
    instructions (the Tile exit drain carries one wait per live sem).
    Split excess waits onto preceding same-engine carrier drains."""
    fn = nc.m.functions[0]
    n = 0
    for blk in fn.blocks:
        out = []
        for ins in blk.instructions:
            si = ins.sync_info
            waits = list(si.on_wait) if (si and si.on_wait) else []
            if len(waits) > max_waits:
                extra, keep = waits[:-max_waits], waits[-max_waits:]
                for i in range(0, len(extra), max_waits):
                    carrier = mybir.InstDrain(
                        name=f"{ins.name}_wsplit{i}", ins=[], outs=[],
                        bass_is_fusable=False)
                    carrier.engine = ins.engine
                    carrier.sync_info = mybir.SyncInfo(
                        on_wait=extra[i:i + max_waits], on_update=[])
                    out.append(carrier)
                    n += 1
                si.on_wait = keep
            out.append(ins)
        blk.instructions = out
    return n


def _build(dist=True):
    nc = bass.Bass(num_devices=NCORES) if dist else bass.Bass()

    aslc_d = nc.dram_tensor("aslc", [KSL, P, N], FP8,
                            kind="ExternalInput")
    if dist:
        aslc_i = nc.dram_tensor("aslc_i", [KSL, P, N], FP8,
                                kind="Internal")
        a_g = nc.dram_tensor("a_g", [NT, P, N], FP8, kind="Internal",
                             addr_space="Shared")
        b_i = nc.dram_tensor("b_i", [KSL, P, N], FP8, kind="Internal")
        b_g = nc.dram_tensor("b_g", [NT, P, N], FP8, kind="Internal",
                             addr_space="Shared")
        wslc_d = nc.dram_tensor("wslc", [WSL], BF16, kind="ExternalInput")
        wslc_i = nc.dram_tensor("wslc_i", [WSL], BF16, kind="Internal")
        w_g = nc.dram_tensor("w_g", [WTOT], BF16, kind="Internal",
                             addr_space="Shared")
    else:
        afull_d = nc.dram_tensor("afull", [NT, P, N], FP8,
                                 kind="ExternalInput")
        bfull_d = nc.dram_tensor("bfull", [NT, P, N], FP8,
                                 kind="ExternalInput")
        wfull_d = nc.dram_tensor("wfull", [WTOT], BF16, kind="ExternalInput")
    xab_d = nc.dram_tensor("xab", [3 * S, N], BF16, kind="ExternalInput")
    bias_d = nc.dram_tensor("bias", [64, 12], F32, kind="ExternalInput")
    out_d = nc.dram_tensor("out", [HOR, N], BF16, kind="ExternalOutput")

    with tile.TileContext(nc) as tc:
        with tc.tile_pool(name="const", bufs=1) as cpool, \
             tc.tile_pool(name="state", bufs=1) as spool, \
             tc.tile_pool(name="psum", bufs=1, space="PSUM") as ppool:

            ab8 = cpool.tile([P, NT, 2 * N], FP8, name="ab8")
            ancol = cpool.tile([P, NT, KSL * P], FP8, name="ancol")
            aslcsb = cpool.tile([P, KSL, N], FP8, name="aslcsb")
            bstg = cpool.tile([P, KSL, N], FP8, name="bstg")
            wsb = {}
            for name, rows, cols, kind in _WLAYOUT:
                if name == "nodeT":
                    continue
                if kind == "bf16":
                    wsb[name] = cpool.tile([rows, cols], BF16, name=name)
                else:
                    wsb[name] = cpool.tile([rows, 2, cols], FP8, name=name)
            bias = cpool.tile([64, 12], F32, name="bias")
            identb = cpool.tile([P, P], BF16, name="identb")
            ident8 = cpool.tile([P, P], FP8, name="ident8")

            h_nat = spool.tile([P, NT, H], FP8, name="h_nat")
            rh_nat = spool.tile([P, NT, H], FP8, name="rh_nat")
            zxT = spool.tile([115, N], BF16, name="zxT")    # h | node | x
            abT = spool.tile([H, 2, N], FP8, name="abT")    # A@h | B@h
            rzxT = spool.tile([115, N], BF16, name="rzxT")  # rh | node | x
            rabT = spool.tile([H, 2, N], FP8, name="rabT")  # A@rh | B@rh
            xabsb = spool.tile([3 * S, N], BF16, name="xabsb")
            cT = spool.tile([H, N], BF16, name="cT")
            tmpT = spool.tile([H, N], BF16, name="tmpT")
            rT = spool.tile([H, N], BF16, name="rT")
            uT = spool.tile([H, N], BF16, name="uT")
            yT = spool.tile([1, N], BF16, name="yT")

            make_identity(nc, identb[:, :])
            nc.vector.tensor_copy(ident8[:, :], identb[:, :])

            # ---------- prologue: gathers + weight loads ----------
            if dist:
                nc.sync.dma_start(wslc_i[:], wslc_d[:])
                nc.gpsimd.collective_compute(
                    "AllGather", mybir.AluOpType.bypass,
                    replica_groups=[list(range(NCORES))],
                    ins=[wslc_i[:]], outs=[w_g[:]])
                nc.sync.dma_start(aslc_i[:, :, :], aslc_d[:, :, :])
                nc.gpsimd.collective_compute(
                    "AllGather", mybir.AluOpType.bypass,
                    replica_groups=[list(range(NCORES))],
                    ins=[aslc_i[:, :, :]], outs=[a_g[:, :, :]])
                wsrc = w_g
            else:
                wsrc = wfull_d

            off = 0
            for name, rows, cols, kind in _WLAYOUT:
                nun = _wsize(rows, cols, kind)
                if kind == "bf16":
                    srcap = wsrc[off:off + nun].rearrange(
                        "(a b) -> a b", a=rows, b=cols)
                else:
                    srcap = wsrc[off:off + nun].bitcast(FP8).rearrange(
                        "(a b c) -> a b c", a=rows, b=2, c=cols)
                if name == "nodeT":
                    nc.sync.dma_start(zxT[64:112, :], srcap)
                    nc.sync.dma_start(rzxT[64:112, :], srcap)
                elif kind == "bf16":
                    nc.sync.dma_start(wsb[name][:, :], srcap)
                else:
                    nc.sync.dma_start(wsb[name][:, :, :], srcap)
                off += nun
            nc.sync.dma_start(bias[:, :], bias_d[:, :])
            for tt in range(KSL):
                nc.sync.dma_start(aslcsb[:, tt, :], aslc_d[tt, :, :])
            asrc = a_g if dist else afull_d
            for k in range(NT):
                nc.sync.dma_start(ab8[:, k, 0:N], asrc[k, :, :])
            nc.sync.dma_start(xabsb[:, :], xab_d[:, :])
            nc.vector.memset(zxT[0:64, :], 0.0)
            nc.vector.memset(rzxT[0:64, :], 0.0)

            # ---------- helpers ----------
            def emit_ancol_build():
                # this core's natural-layout A column-slice = transpose of
                # its own T-slice k-tiles. fp8 PE transpose writes at
                # element step 2, so two blocks pack into one PSUM bank
                # and a single strided copy lands them in ancol.
                for j in range(NT):
                    tp8 = ppool.tile([P, FREE], FP8, name="tp8",
                                     tag="tp", bufs=2)
                    for tt in range(KSL):
                        nc.tensor.transpose(
                            tp8[:, tt * 2 * P:(tt + 1) * 2 * P:2],
                            aslcsb[:, tt, j * P:(j + 1) * P],
                            ident8[:, :])
                    if j % 2 == 0:
                        nc.vector.tensor_copy(ancol[:, j, :], tp8[:, ::2])
                    else:
                        nc.scalar.copy(ancol[:, j, :], tp8[:, ::2])

            def emit_b_build():
                # sharded B-build: this core computes only its KSL B^T
                # tile-rows (stationary = its own natural A column-slice,
                # moving = gathered T tiles), stages them to HBM, and an
                # AllGather assembles the full B in every core's ab8.
                # out_bp[p, f] = SA^2 * B^T[t*128+p, c*512+f]; stored
                # *SB/SA^2 as fp8.
                for tt in range(KSL):
                    bp = [ppool.tile([P, FREE], F32, name=f"bp{c}",
                                     tag="hp", bufs=4) for c in range(NCH)]
                    for l in range(NPAIR):
                        for c in range(NCH):
                            nc.tensor.matmul(
                                bp[c][:, :],
                                ancol[:, 2 * l:2 * l + 2,
                                      tt * P:(tt + 1) * P],
                                ab8[:, 2 * l:2 * l + 2,
                                    c * FREE:(c + 1) * FREE],
                                start=(l == 0), stop=(l == NPAIR - 1),
                                perf_mode=DR)
                    for c in range(NCH):
                        nc.vector.tensor_scalar_mul(
                            bstg[:, tt, c * FREE:(c + 1) * FREE],
                            bp[c][:, :], SB / (SA * SA))
                if dist:
                    for tt in range(KSL):
                        nc.sync.dma_start(b_i[tt, :, :], bstg[:, tt, :])
                    nc.gpsimd.collective_compute(
                        "AllGather", mybir.AluOpType.bypass,
                        replica_groups=[list(range(NCORES))],
                        ins=[b_i[:, :, :]], outs=[b_g[:, :, :]])
                    for t in range(NT):
                        nc.sync.dma_start(ab8[:, t, N:2 * N], b_g[t, :, :])
                else:
                    # core 0's built tiles are t=0..KSL-1: use them so the
                    # sim verifies the on-device transpose+build path
                    for tt in range(KSL):
                        nc.sync.dma_start(ab8[:, tt, N:2 * N],
                                          bstg[:, tt, :])
                    for t in range(KSL, NT):
                        nc.sync.dma_start(ab8[:, t, N:2 * N],
                                          bfull_d[t, :, :])

            def hop_chunk(nat, dstT, c):
                # dstT chunk = [ (A@z).T ; (B@z).T ]. DR matmuls cannot
                # col-tile (ISA), so the two halves accumulate in separate
                # PSUM banks; each k-tile-pair stationary is shared by the
                # A/B matmul pair (consecutive Ldweights dedup).
                hpa = ppool.tile([P, FREE], F32, name="hpa", tag="hp",
                                 bufs=4)
                hpb = ppool.tile([P, FREE], F32, name="hpb", tag="hp",
                                 bufs=4)
                for jp in range(NPAIR):
                    st = nat[:, 2 * jp:2 * jp + 2, :]
                    nc.tensor.matmul(
                        hpa[0:H, :], st,
                        ab8[:, 2 * jp:2 * jp + 2,
                            c * FREE:(c + 1) * FREE],
                        start=(jp == 0), stop=(jp == NPAIR - 1),
                        perf_mode=DR)
                    nc.tensor.matmul(
                        hpb[0:H, :], st,
                        ab8[:, 2 * jp:2 * jp + 2,
                            N + c * FREE:N + (c + 1) * FREE],
                        start=(jp == 0), stop=(jp == NPAIR - 1),
                        perf_mode=DR)
                sl = slice(c * FREE, (c + 1) * FREE)
                # the PSUM halves carry the fp8 A/B scale factors; divide
                # them out here so the fp8 dense weights keep their
                # natural magnitude (W/SA in fp8 would be subnormal).
                # Engines alternate per chunk to balance DVE/Act load.
                if c % 2 == 0:
                    nc.vector.tensor_scalar_mul(dstT[0:H, 0, sl],
                                                hpa[0:H, :], 1.0 / SA)
                    nc.scalar.mul(dstT[0:H, 1, sl], hpb[0:H, :], 1.0 / SB)
                else:
                    nc.scalar.mul(dstT[0:H, 0, sl], hpa[0:H, :], 1.0 / SA)
                    nc.vector.tensor_scalar_mul(dstT[0:H, 1, sl],
                                                hpb[0:H, :], 1.0 / SB)

            def dense_chunk(groups, m, c):
                dp = ppool.tile([P, FREE], F32, name="dp", tag="dp",
                                bufs=2)
                ng = len(groups)
                for gi, (w_ap, rhs, kr, mode) in enumerate(groups):
                    if mode is None:
                        mov = rhs[0:kr, c * FREE:(c + 1) * FREE]
                    else:
                        mov = rhs[0:H, :, c * FREE:(c + 1) * FREE]
                    nc.tensor.matmul(
                        dp[0:m, :], w_ap, mov,
                        start=(gi == 0), stop=(gi == ng - 1),
                        perf_mode=mode)
                return dp

            def to_nat_group(srcT, dst, g):
                # natural fp8 tiles for 4 k-tiles: 4 transposes batched per
                # psum tile, one cast copy
                j0 = 4 * g
                tp = ppool.tile([P, 4 * H], BF16, name="tp", tag="tp",
                                bufs=2)
                for jj in range(4):
                    nc.tensor.transpose(
                        tp[:, jj * H:(jj + 1) * H],
                        srcT[0:H, (j0 + jj) * P:(j0 + jj + 1) * P],
                        identb[0:H, 0:H])
                if g % 2 == 1:
                    nc.vector.tensor_copy(dst[:, j0:j0 + 4, :], tp[:, :])
                else:
                    nc.scalar.copy(dst[:, j0:j0 + 4, :], tp[:, :])

            def make_rh_chunk(c):
                sl = slice(c * FREE, (c + 1) * FREE)
                nc.vector.tensor_mul(rzxT[0:H, sl], rT[:, sl],
                                     zxT[0:H, sl])
                to_nat_group(rzxT, rh_nat, c)

            def update_chunk(c, last):
                # h' = c + u*(h-c)
                sl = slice(c * FREE, (c + 1) * FREE)
                nc.vector.tensor_sub(tmpT[:, sl], zxT[0:H, sl], cT[:, sl])
                nc.vector.tensor_mul(tmpT[:, sl], tmpT[:, sl], uT[:, sl])
                nc.vector.tensor_add(zxT[0:H, sl], tmpT[:, sl], cT[:, sl])
                if not last:
                    to_nat_group(zxT, h_nat, c)

            def gate_chunk(groups, rcol, ucol, with_rh, c):
                dp = dense_chunk(groups, 128, c)
                sl = slice(c * FREE, (c + 1) * FREE)
                nc.scalar.activation(rT[:, sl], dp[0:64, :], AF.Sigmoid,
                                     bias=bias[:, rcol:rcol + 1])
                nc.scalar.activation(uT[:, sl], dp[64:128, :], AF.Sigmoid,
                                     bias=bias[:, ucol:ucol + 1])
                if with_rh:
                    make_rh_chunk(c)

            def cand_chunk(groups, bcol, last, c):
                dp = dense_chunk(groups, 64, c)
                sl = slice(c * FREE, (c + 1) * FREE)
                nc.scalar.activation(cT[:, sl], dp[0:64, :], AF.Tanh,
                                     bias=bias[:, bcol:bcol + 1])
                update_chunk(c, last)

            # ---------- encoder ----------
            def enc_step(t):
                nc.sync.dma_start(zxT[112:115, :], xabsb[3 * t:3 * t + 3, :])
                nc.sync.dma_start(rzxT[112:115, :],
                                  xabsb[3 * t:3 * t + 3, :])
                have_h = t > 0
                # t=0: h == 0 exactly -> A@h/B@h groups contribute +0.0,
                # and the h rows of zxT are memset; drop the ab groups.
                g_groups = [(wsb["wge1"][:, :], zxT, 115, None)]
                c_groups = [(wsb["wce1"][:, :], rzxT, 115, None)]
                if have_h:
                    g_groups.append((wsb["wge2"][:, :, :], abT, None, DR))
                    c_groups.append((wsb["wce2"][:, :, :], rabT, None, DR))
                for c in range(NCH):
                    if have_h:
                        hop_chunk(h_nat, abT, c)
                    if c > 0:
                        gate_chunk(g_groups, 0, 1, have_h, c - 1)
                gate_chunk(g_groups, 0, 1, have_h, NCH - 1)
                for c in range(NCH):
                    if have_h:
                        hop_chunk(rh_nat, rabT, c)
                    if c > 0:
                        cand_chunk(c_groups, 2, False, c - 1)
                cand_chunk(c_groups, 2, False, NCH - 1)

            enc_step(0)
            # A-column + B builds are emitted after the (hop-free) t=0
            # cell so the PE chews t=0's dense work while the A AllGather
            # completes; encoder t=1 hop chains consume B as it lands.
            emit_ancol_build()
            emit_b_build()
            for t in range(1, S):
                enc_step(t)

            # ---------- decoder ----------
            for u in range(HOR):
                wg1 = wsb["wgd01"] if u == 0 else wsb["wgdf1"]
                wg2 = wsb["wgd02"] if u == 0 else wsb["wgdf2"]
                g_groups = [(wg1[:, :], zxT, 64, None),
                            (wg2[:, :, :], abT, None, DR)]
                c_groups = [(wsb["wcd01"][:, :], rzxT, 64, None),
                            (wsb["wcd02"][:, :, :], rabT, None, DR)]
                if u > 0:
                    c_groups = [(wsb["wcdy1"][:, :], zxT, 64, None),
                                (wsb["wcdy2"][:, :, :], abT, None, DR)] \
                        + c_groups
                rc, uc = (3, 4) if u == 0 else (5, 6)
                for c in range(NCH):
                    hop_chunk(h_nat, abT, c)
                    if c > 0:
                        gate_chunk(g_groups, rc, uc, True, c - 1)
                gate_chunk(g_groups, rc, uc, True, NCH - 1)
                bc = 7 if u == 0 else 8
                for c in range(NCH):
                    hop_chunk(rh_nat, rabT, c)
                    if c > 0:
                        cand_chunk(c_groups, bc, u == HOR - 1, c - 1)
                cand_chunk(c_groups, bc, u == HOR - 1, NCH - 1)
                # y = h' @ Wproj + b  (output only; feedback is folded)
                for c in range(NCH):
                    yp = ppool.tile([P, FREE], F32, name="yp", tag="dp",
                                    bufs=2)
                    nc.tensor.matmul(yp[0:1, :], wsb["wproj"][:, :],
                                     zxT[0:H, c * FREE:(c + 1) * FREE],
                                     start=True, stop=True)
                    nc.scalar.activation(yT[0:1, c * FREE:(c + 1) * FREE],
                                         yp[0:1, :], AF.Identity,
                                         bias=bias[0:1, 9:10])
                nc.sync.dma_start(out_d[u:u + 1, :], yT[:, :])

    _dedup_ldweights(nc)
    _split_multiwait(nc)
    return nc


# ---------------- host-side preprocessing ----------------

def _softplus(x):
    return np.log1p(np.exp(-np.abs(x))) + np.maximum(x, 0.0)


def _q8(x):
    # TRN e4m3 overflows to inf above +-240 (unlike OCP's 448): clip first.
    return np.clip(np.asarray(x, np.float32), -240.0, 240.0).astype(
        ml_dtypes.float8_e4m3)


def _host_prep(inp, dist=True):
    """Edge-weight MLP + row-normalization + fp8 A slices + all linearity
    folds. Pure per-graph preprocessing (no time loop). A-natural and
    B = A@A are built on-device; x/node multi-hop rows use exact f32 A."""
    f = np.float32
    bf = ml_dtypes.bfloat16
    row, col = np.asarray(inp["sparse_idx"])
    loc = np.asarray(inp["loc"], f)
    delta = loc[col] - loc[row]
    h1 = np.tanh(delta @ np.asarray(inp["Wk0"], f) + np.asarray(inp["bk0"], f))
    h2 = np.tanh(h1 @ np.asarray(inp["Wk1"], f) + np.asarray(inp["bk1"], f))
    ker = _softplus((h2 @ np.asarray(inp["Wk2"], f)
                     + np.asarray(inp["bk2"], f))[:, 0])
    geo = np.asarray(inp["geodesic"], f)
    w = ker * np.asarray(inp["angle_ratio"], f) * np.exp(-geo * geo)
    denom = np.zeros(N, f)
    np.add.at(denom, row, w)
    w = (w / (denom[row] + np.float32(1e-8))).astype(f)
    A = np.zeros((N, N), f)
    np.add.at(A, (row, col), w)

    # fp8 A: T-layout k-tiles (hop moving operand) and natural row-tiles
    # (B-build stationary): a8t[k, p, m] = A[m, k*128+p] * SA,
    # an8[j, p, i] = A[j*128+p, i] * SA
    a8t = _q8(A.T * SA).reshape(NT, P, N)
    an8 = _q8(A * SA).reshape(NT, P, N)

    Wfe = np.asarray(inp["W_fe"], f)      # (1, 16)
    bfe = np.asarray(inp["b_fe"], f)
    Wp = np.asarray(inp["W_proj"], f)     # (64, 1)
    bp = np.asarray(inp["b_proj"], f)
    node = np.asarray(inp["node_emb"], f)
    SC = [1.0, SA, SB]

    # encoder fold: z rows per hop k are [feat16 | node16 | x1 | h64].
    # x/node hop rows are computed with exact f32 A on the host (no SC).
    def enc_fold(W):
        out = W.shape[1]
        Wx = np.zeros((3, out), f)
        b_extra = np.zeros(out, f)
        Wh = np.zeros((64, 3 * out), f)
        for k in range(3):
            Wk = W[k * 97:(k + 1) * 97]
            Wf, Wxr, Whk = Wk[0:16], Wk[32:33], Wk[33:97]
            Wx[k] = Wxr[0] + Wfe[0] @ Wf
            b_extra += bfe @ Wf
            Wh[:, k * out:(k + 1) * out] = Whk
        return Wx, Wh, b_extra

    Wg_e = np.asarray(inp["Wg_e"], f)
    Wc_e = np.asarray(inp["Wc_e"], f)
    wgx, wge, bg_x = enc_fold(Wg_e)
    wcx, wce, bc_x = enc_fold(Wc_e)
    bg_e = np.asarray(inp["bg_e"], f) + bg_x
    bc_e = np.asarray(inp["bc_e"], f) + bc_x

    # node rhs rows: [node.T; (A node).T; (B node).T] exact f32; per-hop
    # weight blocks stacked in wgn/wcn rows 0-47; rows 48-50 hold the
    # folded x/Ax/Bx weights (rhs rows DMA'd per step)
    Anode = A @ node
    nodeT = np.concatenate([node.T, Anode.T, (A @ Anode).T], axis=0)
    wgn = np.zeros((51, 128), f)
    wcn = np.zeros((51, 64), f)
    for k in range(3):
        wgn[k * 16:(k + 1) * 16] = Wg_e[k * 97 + 16:k * 97 + 32]
        wcn[k * 16:(k + 1) * 16] = Wc_e[k * 97 + 16:k * 97 + 32]
    wgn[48:51] = wgx
    wcn[48:51] = wcx

    # decoder fold: z rows per hop k are [y1 | h64]
    Wg_d = np.asarray(inp["Wg_d"], f)
    Wc_d = np.asarray(inp["Wc_d"], f)

    def dec_fold(W):
        out = W.shape[1]
        Wh_plain = np.zeros((64, 3 * out), f)
        Wh_fold = np.zeros((64, 3 * out), f)
        Wy_h = np.zeros((64, 3 * out), f)
        b_extra = np.zeros(out, f)
        for k in range(3):
            Wk = W[k * 65:(k + 1) * 65]
            Wy, Wh = Wk[0:1], Wk[1:65]
            Wh_plain[:, k * out:(k + 1) * out] = Wh
            Wh_fold[:, k * out:(k + 1) * out] = Wh + Wp @ Wy
            Wy_h[:, k * out:(k + 1) * out] = Wp @ Wy
            b_extra += bp @ Wy
        return Wh_plain, Wh_fold, Wy_h, b_extra

    wgd0, wgdf, _, bgd_x = dec_fold(Wg_d)
    wcd0, _, wcdy, bcd_x = dec_fold(Wc_d)
    bg_d = np.asarray(inp["bg_d"], f)
    bc_d = np.asarray(inp["bc_d"], f)

    bias = np.zeros((64, 12), f)
    bias[:, 0] = bg_e[0:64]
    bias[:, 1] = bg_e[64:128]
    bias[:, 2] = bc_e
    bias[:, 3] = bg_d[0:64]
    bias[:, 4] = bg_d[64:128]
    bias[:, 5] = (bg_d + bgd_x)[0:64]
    bias[:, 6] = (bg_d + bgd_x)[64:128]
    bias[:, 7] = bc_d
    bias[:, 8] = bc_d + bcd_x
    bias[0, 9] = bp[0]

    # pack the dense-weight blob in _WLAYOUT order:
    # group 1 rhs is [h | node | x] (K=115), group 2 rhs is [A@z|B@z]
    wparts = {
        "nodeT": nodeT,
        "wge1": np.concatenate([wge[:, 0:128], wgn]),
        "wge2": np.stack([wge[:, 128:256], wge[:, 256:384]], axis=1),
        "wce1": np.concatenate([wce[:, 0:64], wcn]),
        "wce2": np.stack([wce[:, 64:128], wce[:, 128:192]], axis=1),
        "wgd01": wgd0[:, 0:128],
        "wgd02": np.stack([wgd0[:, 128:256], wgd0[:, 256:384]], axis=1),
        "wgdf1": wgdf[:, 0:128],
        "wgdf2": np.stack([wgdf[:, 128:256], wgdf[:, 256:384]], axis=1),
        "wcd01": wcd0[:, 0:64],
        "wcd02": np.stack([wcd0[:, 64:128], wcd0[:, 128:192]], axis=1),
        "wcdy1": wcdy[:, 0:64],
        "wcdy2": np.stack([wcdy[:, 64:128], wcdy[:, 128:192]], axis=1),
        "wproj": Wp,
    }
    segs = []
    for nm, r, c, kind in _WLAYOUT:
        part = np.ascontiguousarray(wparts[nm])
        if kind == "bf16":
            assert part.shape == (r, c), (nm, part.shape)
            segs.append(part.astype(bf).ravel())
        else:
            assert part.shape == (r, 2, c), (nm, part.shape)
            raw = _q8(part).ravel().view(np.uint8)
            segs.append(raw.view(bf))
    blob = np.concatenate(segs)
    assert blob.shape == (WTOT,)

    shared = {"bias": bias}
    if not dist:
        shared["afull"] = a8t
        # reference fp8 B exactly as the device build would produce it
        A8m = an8.astype(f).reshape(N, N) / SA
        Bq = A8m @ A8m
        shared["bfull"] = _q8(Bq.T * SB).reshape(NT, P, N)
        shared["wfull"] = blob

    xs = np.asarray(inp["inputs"], f)[:, :, :, 0]    # (S, B, N)
    in_maps = []
    for b in range(NCORES):
        X = xs[:, b, :]                              # (S, N)
        AXt = X @ A.T                                # exact f32 (A@x).T rows
        BXt = AXt @ A.T
        xab = np.stack([X, AXt, BXt])                # (3, S, N)
        m = dict(shared)
        # rows (t, k): [x_t; (A@x_t); (B@x_t)] contiguous per step
        m["xab"] = np.ascontiguousarray(
            xab.transpose(1, 0, 2).reshape(3 * S, N)).astype(bf)
        m["aslc"] = np.ascontiguousarray(a8t[KSL * b:KSL * (b + 1)])
        if dist:
            m["wslc"] = np.ascontiguousarray(blob[WSL * b:WSL * (b + 1)])
        in_maps.append(m)
    return in_maps


_NC_CACHE = []


def kernel(**inputs):
    if not _NC_CACHE:
        _NC_CACHE.append(_build())
    nc = _NC_CACHE[0]
    in_maps = _host_prep(inputs)
    res = run_bass_kernel_spmd(nc, in_maps, core_ids=list(range(NCORES)))
    out = np.stack([res.results[b]["out"] for b in range(NCORES)], axis=1)
    return np.ascontiguousarray(out[..., None].astype(np.float32))


# revision 24
# speedup vs baseline: 1.2184x; 1.2184x over previous
"""Trainium2 Bass kernel for nn_CLCRNModel (CLCRN encoder-decoder GNN).

Strategy: data-parallel over batch (8 batch elements -> 8 NeuronCores).
The sparse 25-neighbor graph conv is cast as dense matmuls against the
row-normalized adjacency A and its square B = A^2, both SBUF-resident in
fp8-e4m3 and streamed through the PE with DoubleRow (2 fp8 MACs/cell).

Input-volume optimization: every core receives only a 2-k-tile slice of
A^T (512 KB fp8) plus a 1/8 slice of a packed weight blob; the full A is
assembled on device with an HBM AllGather, its natural-layout copy is
built with PE transposes, and B = A@A is computed on the PE (fp8 DR)
directly into the SBUF hop operand. No dense matrix crosses the
host-device link.

Per cell the PE runs two fused hop passes -- [A@z | B@z] accumulate into
disjoint halves of one PSUM bank per 512-chunk, so each k-tile-pair's
stationary is loaded once per pass (links-outer) and each chunk needs a
single PSUM->SBUF copy. Dense gate/cand matmuls are grouped as
[h|node|x] (bf16, K=115) and [A@z|B@z] (K=128) moving operands.

Host-side linear-algebra folds shrink every hop pass to the 64 hidden
channels: encoder feature-embedding and node embedding fold into
precomputed dense rows/biases (exact f32 A), and the decoder feedback
y_t = h_t @ W_proj + b_proj folds into the h-group dense weights.
"""
import os
import sys

for _p in ("/root/.axon_site/_ro/trn_rl_repo", "/opt/trn_rl_repo"):
    if os.path.isdir(_p) and _p not in sys.path:
        sys.path.append(_p)

import numpy as np
import ml_dtypes

import concourse.bass as bass
import concourse.mybir as mybir
import concourse.tile as tile
from concourse.bass_utils import run_bass_kernel_spmd
from concourse.masks import make_identity

P = 128
N = 2048
NT = 16            # node k-tiles
NPAIR = 8          # DoubleRow k-tile pairs
S = 12             # encoder steps
HOR = 12           # decoder steps
H = 64             # GRU units
FREE = 512         # hop chunk width (fp8 DR moving limit: 2x512)
NCH = N // FREE
NCORES = 8
KSL = NT // NCORES  # k-tiles per core slice
SA = 16.0          # fp8 scale for A
SB = 128.0         # fp8 scale for B (= SA^2 * 0.5, applied in B-build copy)

F32 = mybir.dt.float32
BF16 = mybir.dt.bfloat16
FP8 = mybir.dt.float8e4
AF = mybir.ActivationFunctionType
DR = mybir.MatmulPerfMode.DoubleRow

# packed weight blob (bf16-unit offsets). kind "bf16": [rows, cols] bf16
# tile; kind "fp8": [64, 2, cols] fp8 DoubleRow-paired tile, stored as raw
# byte pairs inside the bf16 blob (AP.bitcast on device).
# nodeT is [node; A@node; B@node].T
_WLAYOUT = [
    ("nodeT", 48, N, "bf16"),
    ("wge1", 115, 128, "bf16"), ("wge2", 64, 128, "fp8"),
    ("wce1", 115, 64, "bf16"), ("wce2", 64, 64, "fp8"),
    ("wgd01", 64, 128, "bf16"), ("wgd02", 64, 128, "fp8"),
    ("wgdf1", 64, 128, "bf16"), ("wgdf2", 64, 128, "fp8"),
    ("wcd01", 64, 64, "bf16"), ("wcd02", 64, 64, "fp8"),
    ("wcdy1", 64, 64, "bf16"), ("wcdy2", 64, 64, "fp8"),
    ("wproj", 64, 1, "bf16"),
]


def _wsize(r, c, kind):
    # bf16-unit count in the blob: fp8 parts are [r, 2, c] fp8 = r*c units
    return r * c if kind == "bf16" else r * c


WTOT = sum(_wsize(r, c, k) for _, r, c, k in _WLAYOUT)
assert WTOT % NCORES == 0
WSL = WTOT // NCORES


def _dedup_ldweights(nc):
    """Remove Ldweights whose weights AP equals the previous PE weight
    load (PE retains the stationary operand between matmuls; walrus's own
    ldw-opt is disabled in this toolchain). Waits/updates of a removed
    load migrate to the next PE instruction."""
    import concourse.mybir as _mb
    fn = nc.m.functions[0]
    pe = _mb.EngineType.PE
    n = 0
    for blk in fn.blocks:
        out = []
        last_sig = None
        pend_waits, pend_updates = [], []
        for ins in blk.instructions:
            if ins.engine == pe:
                if ins.opcode == "Ldweights":
                    sig = (str(ins.ins[0]), str(ins.tile_position),
                           str(ins.perf_mode), str(ins.is_transpose))
                    if sig == last_sig:
                        si = ins.sync_info
                        if si:
                            pend_waits.extend(si.on_wait or [])
                            pend_updates.extend(si.on_update or [])
                        n += 1
                        continue
                    last_sig = sig
                elif ins.opcode not in ("Matmult", "Drain", "EventSemaphore",
                                        "RegisterMove", "UnconditionalBranch"):
                    last_sig = None
                if pend_waits or pend_updates:
                    si = ins.sync_info
                    if si is None:
                        si = _mb.SyncInfo(on_wait=[], on_update=[])
                        ins.sync_info = si
                    si.on_wait = list(pend_waits) + list(si.on_wait or [])
                    si.on_update = list(si.on_update or []) + list(pend_updates)
                    pend_waits, pend_updates = [], []
            out.append(ins)
        assert not pend_waits and not pend_updates
        blk.instructions = out
    return n


def _split_multiwait(nc, max_waits=1):
    """This container's walrus rejects >1 sem-wait on CTRL-class
    instructions (the Tile exit drain carries one wait per live sem).
    Split excess waits onto preceding same-engine carrier drains."""
    fn = nc.m.functions[0]
    n = 0
    for blk in fn.blocks:
        out = []
        for ins in blk.instructions:
            si = ins.sync_info
            waits = list(si.on_wait) if (si and si.on_wait) else []
            if len(waits) > max_waits:
                extra, keep = waits[:-max_waits], waits[-max_waits:]
                for i in range(0, len(extra), max_waits):
                    carrier = mybir.InstDrain(
                        name=f"{ins.name}_wsplit{i}", ins=[], outs=[],
                        bass_is_fusable=False)
                    carrier.engine = ins.engine
                    carrier.sync_info = mybir.SyncInfo(
                        on_wait=extra[i:i + max_waits], on_update=[])
                    out.append(carrier)
                    n += 1
                si.on_wait = keep
            out.append(ins)
        blk.instructions = out
    return n


def _build(dist=True):
    nc = bass.Bass(num_devices=NCORES) if dist else bass.Bass()

    aslc_d = nc.dram_tensor("aslc", [KSL, P, N], FP8,
                            kind="ExternalInput")
    if dist:
        aslc_i = nc.dram_tensor("aslc_i", [KSL, P, N], FP8,
                                kind="Internal")
        a_g = nc.dram_tensor("a_g", [NT, P, N], FP8, kind="Internal",
                             addr_space="Shared")
        b_i = nc.dram_tensor("b_i", [KSL, P, N], FP8, kind="Internal")
        b_g = nc.dram_tensor("b_g", [NT, P, N], FP8, kind="Internal",
                             addr_space="Shared")
        wslc_d = nc.dram_tensor("wslc", [WSL], BF16, kind="ExternalInput")
        wslc_i = nc.dram_tensor("wslc_i", [WSL], BF16, kind="Internal")
        w_g = nc.dram_tensor("w_g", [WTOT], BF16, kind="Internal",
                             addr_space="Shared")
    else:
        afull_d = nc.dram_tensor("afull", [NT, P, N], FP8,
                                 kind="ExternalInput")
        bfull_d = nc.dram_tensor("bfull", [NT, P, N], FP8,
                                 kind="ExternalInput")
        wfull_d = nc.dram_tensor("wfull", [WTOT], BF16, kind="ExternalInput")
    xab_d = nc.dram_tensor("xab", [3 * S, N], BF16, kind="ExternalInput")
    bias_d = nc.dram_tensor("bias", [64, 12], F32, kind="ExternalInput")
    out_d = nc.dram_tensor("out", [HOR, N], BF16, kind="ExternalOutput")

    with tile.TileContext(nc) as tc:
        with tc.tile_pool(name="const", bufs=1) as cpool, \
             tc.tile_pool(name="state", bufs=1) as spool, \
             tc.tile_pool(name="psum", bufs=1, space="PSUM") as ppool:

            ab8 = cpool.tile([P, NT, 2 * N], FP8, name="ab8")
            ancol = cpool.tile([P, NT, KSL * P], FP8, name="ancol")
            aslcsb = cpool.tile([P, KSL, N], FP8, name="aslcsb")
            bstg = cpool.tile([P, KSL, N], FP8, name="bstg")
            wsb = {}
            for name, rows, cols, kind in _WLAYOUT:
                if name == "nodeT":
                    continue
                if kind == "bf16":
                    wsb[name] = cpool.tile([rows, cols], BF16, name=name)
                else:
                    wsb[name] = cpool.tile([rows, 2, cols], FP8, name=name)
            bias = cpool.tile([64, 12], F32, name="bias")
            identb = cpool.tile([P, P], BF16, name="identb")
            ident8 = cpool.tile([P, P], FP8, name="ident8")

            h_nat = spool.tile([P, NT, H], FP8, name="h_nat")
            rh_nat = spool.tile([P, NT, H], FP8, name="rh_nat")
            zxT = spool.tile([115, N], BF16, name="zxT")    # h | node | x
            abT = spool.tile([H, 2, N], FP8, name="abT")    # A@h | B@h
            rzxT = spool.tile([115, N], BF16, name="rzxT")  # rh | node | x
            rabT = spool.tile([H, 2, N], FP8, name="rabT")  # A@rh | B@rh
            xabsb = spool.tile([3 * S, N], BF16, name="xabsb")
            cT = spool.tile([H, N], BF16, name="cT")
            tmpT = spool.tile([H, N], BF16, name="tmpT")
            rT = spool.tile([H, N], BF16, name="rT")
            uT = spool.tile([H, N], BF16, name="uT")
            yT = spool.tile([1, N], BF16, name="yT")

            make_identity(nc, identb[:, :])
            nc.vector.tensor_copy(ident8[:, :], identb[:, :])

            # ---------- prologue: gathers + weight loads ----------
            if dist:
                nc.sync.dma_start(aslc_i[:, :, :], aslc_d[:, :, :])
                nc.gpsimd.collective_compute(
                    "AllGather", mybir.AluOpType.bypass,
                    replica_groups=[list(range(NCORES))],
                    ins=[aslc_i[:, :, :]], outs=[a_g[:, :, :]])
                nc.sync.dma_start(wslc_i[:], wslc_d[:])
                nc.gpsimd.collective_compute(
                    "AllGather", mybir.AluOpType.bypass,
                    replica_groups=[list(range(NCORES))],
                    ins=[wslc_i[:]], outs=[w_g[:]])
                wsrc = w_g
            else:
                wsrc = wfull_d

            off = 0
            for name, rows, cols, kind in _WLAYOUT:
                nun = _wsize(rows, cols, kind)
                if kind == "bf16":
                    srcap = wsrc[off:off + nun].rearrange(
                        "(a b) -> a b", a=rows, b=cols)
                else:
                    srcap = wsrc[off:off + nun].bitcast(FP8).rearrange(
                        "(a b c) -> a b c", a=rows, b=2, c=cols)
                if name == "nodeT":
                    nc.sync.dma_start(zxT[64:112, :], srcap)
                    nc.sync.dma_start(rzxT[64:112, :], srcap)
                elif kind == "bf16":
                    nc.sync.dma_start(wsb[name][:, :], srcap)
                else:
                    nc.sync.dma_start(wsb[name][:, :, :], srcap)
                off += nun
            nc.sync.dma_start(bias[:, :], bias_d[:, :])
            for tt in range(KSL):
                nc.sync.dma_start(aslcsb[:, tt, :], aslc_d[tt, :, :])
            asrc = a_g if dist else afull_d
            for k in range(NT):
                nc.sync.dma_start(ab8[:, k, 0:N], asrc[k, :, :])
            nc.sync.dma_start(xabsb[:, :], xab_d[:, :])
            nc.vector.memset(zxT[0:64, :], 0.0)
            nc.vector.memset(rzxT[0:64, :], 0.0)

            # ---------- helpers ----------
            def emit_ancol_build():
                # this core's natural-layout A column-slice = transpose of
                # its own T-slice k-tiles. fp8 PE transpose writes at
                # element step 2, so two blocks pack into one PSUM bank
                # and a single strided copy lands them in ancol.
                for j in range(NT):
                    tp8 = ppool.tile([P, FREE], FP8, name="tp8",
                                     tag="tp", bufs=2)
                    for tt in range(KSL):
                        nc.tensor.transpose(
                            tp8[:, tt * 2 * P:(tt + 1) * 2 * P:2],
                            aslcsb[:, tt, j * P:(j + 1) * P],
                            ident8[:, :])
                    if j % 2 == 0:
                        nc.vector.tensor_copy(ancol[:, j, :], tp8[:, ::2])
                    else:
                        nc.scalar.copy(ancol[:, j, :], tp8[:, ::2])

            def emit_b_build():
                # sharded B-build: this core computes only its KSL B^T
                # tile-rows (stationary = its own natural A column-slice,
                # moving = gathered T tiles), stages them to HBM, and an
                # AllGather assembles the full B in every core's ab8.
                # out_bp[p, f] = SA^2 * B^T[t*128+p, c*512+f]; stored
                # *SB/SA^2 as fp8.
                for tt in range(KSL):
                    bp = [ppool.tile([P, FREE], F32, name=f"bp{c}",
                                     tag="hp", bufs=4) for c in range(NCH)]
                    for l in range(NPAIR):
                        for c in range(NCH):
                            nc.tensor.matmul(
                                bp[c][:, :],
                                ancol[:, 2 * l:2 * l + 2,
                                      tt * P:(tt + 1) * P],
                                ab8[:, 2 * l:2 * l + 2,
                                    c * FREE:(c + 1) * FREE],
                                start=(l == 0), stop=(l == NPAIR - 1),
                                perf_mode=DR)
                    for c in range(NCH):
                        nc.vector.tensor_scalar_mul(
                            bstg[:, tt, c * FREE:(c + 1) * FREE],
                            bp[c][:, :], SB / (SA * SA))
                if dist:
                    for tt in range(KSL):
                        nc.sync.dma_start(b_i[tt, :, :], bstg[:, tt, :])
                    nc.gpsimd.collective_compute(
                        "AllGather", mybir.AluOpType.bypass,
                        replica_groups=[list(range(NCORES))],
                        ins=[b_i[:, :, :]], outs=[b_g[:, :, :]])
                    for t in range(NT):
                        nc.sync.dma_start(ab8[:, t, N:2 * N], b_g[t, :, :])
                else:
                    # core 0's built tiles are t=0..KSL-1: use them so the
                    # sim verifies the on-device transpose+build path
                    for tt in range(KSL):
                        nc.sync.dma_start(ab8[:, tt, N:2 * N],
                                          bstg[:, tt, :])
                    for t in range(KSL, NT):
                        nc.sync.dma_start(ab8[:, t, N:2 * N],
                                          bfull_d[t, :, :])

            def hop_chunk(nat, dstT, c):
                # dstT chunk = [ (A@z).T ; (B@z).T ]. DR matmuls cannot
                # col-tile (ISA), so the two halves accumulate in separate
                # PSUM banks; each k-tile-pair stationary is shared by the
                # A/B matmul pair (consecutive Ldweights dedup).
                hpa = ppool.tile([P, FREE], F32, name="hpa", tag="hp",
                                 bufs=4)
                hpb = ppool.tile([P, FREE], F32, name="hpb", tag="hp",
                                 bufs=4)
                for jp in range(NPAIR):
                    st = nat[:, 2 * jp:2 * jp + 2, :]
                    nc.tensor.matmul(
                        hpa[0:H, :], st,
                        ab8[:, 2 * jp:2 * jp + 2,
                            c * FREE:(c + 1) * FREE],
                        start=(jp == 0), stop=(jp == NPAIR - 1),
                        perf_mode=DR)
                    nc.tensor.matmul(
                        hpb[0:H, :], st,
                        ab8[:, 2 * jp:2 * jp + 2,
                            N + c * FREE:N + (c + 1) * FREE],
                        start=(jp == 0), stop=(jp == NPAIR - 1),
                        perf_mode=DR)
                sl = slice(c * FREE, (c + 1) * FREE)
                # the PSUM halves carry the fp8 A/B scale factors; divide
                # them out here so the fp8 dense weights keep their
                # natural magnitude (W/SA in fp8 would be subnormal).
                # Engines alternate per chunk to balance DVE/Act load.
                if c % 2 == 0:
                    nc.vector.tensor_scalar_mul(dstT[0:H, 0, sl],
                                                hpa[0:H, :], 1.0 / SA)
                    nc.scalar.mul(dstT[0:H, 1, sl], hpb[0:H, :], 1.0 / SB)
                else:
                    nc.scalar.mul(dstT[0:H, 0, sl], hpa[0:H, :], 1.0 / SA)
                    nc.vector.tensor_scalar_mul(dstT[0:H, 1, sl],
                                                hpb[0:H, :], 1.0 / SB)

            def dense_chunk(groups, m, c):
                dp = ppool.tile([P, FREE], F32, name="dp", tag="dp",
                                bufs=2)
                ng = len(groups)
                for gi, (w_ap, rhs, kr, mode) in enumerate(groups):
                    if mode is None:
                        mov = rhs[0:kr, c * FREE:(c + 1) * FREE]
                    else:
                        mov = rhs[0:H, :, c * FREE:(c + 1) * FREE]
                    nc.tensor.matmul(
                        dp[0:m, :], w_ap, mov,
                        start=(gi == 0), stop=(gi == ng - 1),
                        perf_mode=mode)
                return dp

            def to_nat_group(srcT, dst, g):
                # natural fp8 tiles for 4 k-tiles: 4 transposes batched per
                # psum tile, one cast copy
                j0 = 4 * g
                tp = ppool.tile([P, 4 * H], BF16, name="tp", tag="tp",
                                bufs=2)
                for jj in range(4):
                    nc.tensor.transpose(
                        tp[:, jj * H:(jj + 1) * H],
                        srcT[0:H, (j0 + jj) * P:(j0 + jj + 1) * P],
                        identb[0:H, 0:H])
                if g % 2 == 1:
                    nc.vector.tensor_copy(dst[:, j0:j0 + 4, :], tp[:, :])
                else:
                    nc.scalar.copy(dst[:, j0:j0 + 4, :], tp[:, :])

            def make_rh_chunk(c):
                sl = slice(c * FREE, (c + 1) * FREE)
                nc.vector.tensor_mul(rzxT[0:H, sl], rT[:, sl],
                                     zxT[0:H, sl])
                to_nat_group(rzxT, rh_nat, c)

            def update_chunk(c, last):
                # h' = c + u*(h-c)
                sl = slice(c * FREE, (c + 1) * FREE)
                nc.vector.tensor_sub(tmpT[:, sl], zxT[0:H, sl], cT[:, sl])
                nc.vector.tensor_mul(tmpT[:, sl], tmpT[:, sl], uT[:, sl])
                nc.vector.tensor_add(zxT[0:H, sl], tmpT[:, sl], cT[:, sl])
                if not last:
                    to_nat_group(zxT, h_nat, c)

            def gate_chunk(groups, rcol, ucol, with_rh, c):
                dp = dense_chunk(groups, 128, c)
                sl = slice(c * FREE, (c + 1) * FREE)
                nc.scalar.activation(rT[:, sl], dp[0:64, :], AF.Sigmoid,
                                     bias=bias[:, rcol:rcol + 1])
                nc.scalar.activation(uT[:, sl], dp[64:128, :], AF.Sigmoid,
                                     bias=bias[:, ucol:ucol + 1])
                if with_rh:
                    make_rh_chunk(c)

            def cand_chunk(groups, bcol, last, c):
                dp = dense_chunk(groups, 64, c)
                sl = slice(c * FREE, (c + 1) * FREE)
                nc.scalar.activation(cT[:, sl], dp[0:64, :], AF.Tanh,
                                     bias=bias[:, bcol:bcol + 1])
                update_chunk(c, last)

            # ---------- encoder ----------
            def enc_step(t):
                nc.sync.dma_start(zxT[112:115, :], xabsb[3 * t:3 * t + 3, :])
                nc.sync.dma_start(rzxT[112:115, :],
                                  xabsb[3 * t:3 * t + 3, :])
                have_h = t > 0
                # t=0: h == 0 exactly -> A@h/B@h groups contribute +0.0,
                # and the h rows of zxT are memset; drop the ab groups.
                g_groups = [(wsb["wge1"][:, :], zxT, 115, None)]
                c_groups = [(wsb["wce1"][:, :], rzxT, 115, None)]
                if have_h:
                    g_groups.append((wsb["wge2"][:, :, :], abT, None, DR))
                    c_groups.append((wsb["wce2"][:, :, :], rabT, None, DR))
                for c in range(NCH):
                    if have_h:
                        hop_chunk(h_nat, abT, c)
                    if c > 0:
                        gate_chunk(g_groups, 0, 1, have_h, c - 1)
                gate_chunk(g_groups, 0, 1, have_h, NCH - 1)
                for c in range(NCH):
                    if have_h:
                        hop_chunk(rh_nat, rabT, c)
                    if c > 0:
                        cand_chunk(c_groups, 2, False, c - 1)
                cand_chunk(c_groups, 2, False, NCH - 1)

            # A-column + B builds are emitted before the (hop-free) t=0
            # cell: the B AllGather chain is the startup critical path,
            # while t=0's dense work overlaps the B exchange.
            emit_ancol_build()
            emit_b_build()
            enc_step(0)
            for t in range(1, S):
                enc_step(t)

            # ---------- decoder ----------
            for u in range(HOR):
                wg1 = wsb["wgd01"] if u == 0 else wsb["wgdf1"]
                wg2 = wsb["wgd02"] if u == 0 else wsb["wgdf2"]
                g_groups = [(wg1[:, :], zxT, 64, None),
                            (wg2[:, :, :], abT, None, DR)]
                c_groups = [(wsb["wcd01"][:, :], rzxT, 64, None),
                            (wsb["wcd02"][:, :, :], rabT, None, DR)]
                if u > 0:
                    c_groups = [(wsb["wcdy1"][:, :], zxT, 64, None),
                                (wsb["wcdy2"][:, :, :], abT, None, DR)] \
                        + c_groups
                rc, uc = (3, 4) if u == 0 else (5, 6)
                for c in range(NCH):
                    hop_chunk(h_nat, abT, c)
                    if c > 0:
                        gate_chunk(g_groups, rc, uc, True, c - 1)
                gate_chunk(g_groups, rc, uc, True, NCH - 1)
                bc = 7 if u == 0 else 8
                for c in range(NCH):
                    hop_chunk(rh_nat, rabT, c)
                    if c > 0:
                        cand_chunk(c_groups, bc, u == HOR - 1, c - 1)
                cand_chunk(c_groups, bc, u == HOR - 1, NCH - 1)
                # y = h' @ Wproj + b  (output only; feedback is folded)
                for c in range(NCH):
                    yp = ppool.tile([P, FREE], F32, name="yp", tag="dp",
                                    bufs=2)
                    nc.tensor.matmul(yp[0:1, :], wsb["wproj"][:, :],
                                     zxT[0:H, c * FREE:(c + 1) * FREE],
                                     start=True, stop=True)
                    nc.scalar.activation(yT[0:1, c * FREE:(c + 1) * FREE],
                                         yp[0:1, :], AF.Identity,
                                         bias=bias[0:1, 9:10])
                nc.sync.dma_start(out_d[u:u + 1, :], yT[:, :])

    _dedup_ldweights(nc)
    _split_multiwait(nc)
    return nc


# ---------------- host-side preprocessing ----------------

def _softplus(x):
    return np.log1p(np.exp(-np.abs(x))) + np.maximum(x, 0.0)


def _q8(x):
    # TRN e4m3 overflows to inf above +-240 (unlike OCP's 448): clip first.
    return np.clip(np.asarray(x, np.float32), -240.0, 240.0).astype(
        ml_dtypes.float8_e4m3)


def _host_prep(inp, dist=True):
    """Edge-weight MLP + row-normalization + fp8 A slices + all linearity
    folds. Pure per-graph preprocessing (no time loop). A-natural and
    B = A@A are built on-device; x/node multi-hop rows use exact f32 A."""
    f = np.float32
    bf = ml_dtypes.bfloat16
    row, col = np.asarray(inp["sparse_idx"])
    loc = np.asarray(inp["loc"], f)
    delta = loc[col] - loc[row]
    h1 = np.tanh(delta @ np.asarray(inp["Wk0"], f) + np.asarray(inp["bk0"], f))
    h2 = np.tanh(h1 @ np.asarray(inp["Wk1"], f) + np.asarray(inp["bk1"], f))
    ker = _softplus((h2 @ np.asarray(inp["Wk2"], f)
                     + np.asarray(inp["bk2"], f))[:, 0])
    geo = np.asarray(inp["geodesic"], f)
    w = ker * np.asarray(inp["angle_ratio"], f) * np.exp(-geo * geo)
    denom = np.zeros(N, f)
    np.add.at(denom, row, w)
    w = (w / (denom[row] + np.float32(1e-8))).astype(f)
    A = np.zeros((N, N), f)
    np.add.at(A, (row, col), w)

    # fp8 A: T-layout k-tiles (hop moving operand) and natural row-tiles
    # (B-build stationary): a8t[k, p, m] = A[m, k*128+p] * SA,
    # an8[j, p, i] = A[j*128+p, i] * SA
    a8t = _q8(A.T * SA).reshape(NT, P, N)
    an8 = _q8(A * SA).reshape(NT, P, N)

    Wfe = np.asarray(inp["W_fe"], f)      # (1, 16)
    bfe = np.asarray(inp["b_fe"], f)
    Wp = np.asarray(inp["W_proj"], f)     # (64, 1)
    bp = np.asarray(inp["b_proj"], f)
    node = np.asarray(inp["node_emb"], f)
    SC = [1.0, SA, SB]

    # encoder fold: z rows per hop k are [feat16 | node16 | x1 | h64].
    # x/node hop rows are computed with exact f32 A on the host (no SC).
    def enc_fold(W):
        out = W.shape[1]
        Wx = np.zeros((3, out), f)
        b_extra = np.zeros(out, f)
        Wh = np.zeros((64, 3 * out), f)
        for k in range(3):
            Wk = W[k * 97:(k + 1) * 97]
            Wf, Wxr, Whk = Wk[0:16], Wk[32:33], Wk[33:97]
            Wx[k] = Wxr[0] + Wfe[0] @ Wf
            b_extra += bfe @ Wf
            Wh[:, k * out:(k + 1) * out] = Whk
        return Wx, Wh, b_extra

    Wg_e = np.asarray(inp["Wg_e"], f)
    Wc_e = np.asarray(inp["Wc_e"], f)
    wgx, wge, bg_x = enc_fold(Wg_e)
    wcx, wce, bc_x = enc_fold(Wc_e)
    bg_e = np.asarray(inp["bg_e"], f) + bg_x
    bc_e = np.asarray(inp["bc_e"], f) + bc_x

    # node rhs rows: [node.T; (A node).T; (B node).T] exact f32; per-hop
    # weight blocks stacked in wgn/wcn rows 0-47; rows 48-50 hold the
    # folded x/Ax/Bx weights (rhs rows DMA'd per step)
    Anode = A @ node
    nodeT = np.concatenate([node.T, Anode.T, (A @ Anode).T], axis=0)
    wgn = np.zeros((51, 128), f)
    wcn = np.zeros((51, 64), f)
    for k in range(3):
        wgn[k * 16:(k + 1) * 16] = Wg_e[k * 97 + 16:k * 97 + 32]
        wcn[k * 16:(k + 1) * 16] = Wc_e[k * 97 + 16:k * 97 + 32]
    wgn[48:51] = wgx
    wcn[48:51] = wcx

    # decoder fold: z rows per hop k are [y1 | h64]
    Wg_d = np.asarray(inp["Wg_d"], f)
    Wc_d = np.asarray(inp["Wc_d"], f)

    def dec_fold(W):
        out = W.shape[1]
        Wh_plain = np.zeros((64, 3 * out), f)
        Wh_fold = np.zeros((64, 3 * out), f)
        Wy_h = np.zeros((64, 3 * out), f)
        b_extra = np.zeros(out, f)
        for k in range(3):
            Wk = W[k * 65:(k + 1) * 65]
            Wy, Wh = Wk[0:1], Wk[1:65]
            Wh_plain[:, k * out:(k + 1) * out] = Wh
            Wh_fold[:, k * out:(k + 1) * out] = Wh + Wp @ Wy
            Wy_h[:, k * out:(k + 1) * out] = Wp @ Wy
            b_extra += bp @ Wy
        return Wh_plain, Wh_fold, Wy_h, b_extra

    wgd0, wgdf, _, bgd_x = dec_fold(Wg_d)
    wcd0, _, wcdy, bcd_x = dec_fold(Wc_d)
    bg_d = np.asarray(inp["bg_d"], f)
    bc_d = np.asarray(inp["bc_d"], f)

    bias = np.zeros((64, 12), f)
    bias[:, 0] = bg_e[0:64]
    bias[:, 1] = bg_e[64:128]
    bias[:, 2] = bc_e
    bias[:, 3] = bg_d[0:64]
    bias[:, 4] = bg_d[64:128]
    bias[:, 5] = (bg_d + bgd_x)[0:64]
    bias[:, 6] = (bg_d + bgd_x)[64:128]
    bias[:, 7] = bc_d
    bias[:, 8] = bc_d + bcd_x
    bias[0, 9] = bp[0]

    # pack the dense-weight blob in _WLAYOUT order:
    # group 1 rhs is [h | node | x] (K=115), group 2 rhs is [A@z|B@z]
    wparts = {
        "nodeT": nodeT,
        "wge1": np.concatenate([wge[:, 0:128], wgn]),
        "wge2": np.stack([wge[:, 128:256], wge[:, 256:384]], axis=1),
        "wce1": np.concatenate([wce[:, 0:64], wcn]),
        "wce2": np.stack([wce[:, 64:128], wce[:, 128:192]], axis=1),
        "wgd01": wgd0[:, 0:128],
        "wgd02": np.stack([wgd0[:, 128:256], wgd0[:, 256:384]], axis=1),
        "wgdf1": wgdf[:, 0:128],
        "wgdf2": np.stack([wgdf[:, 128:256], wgdf[:, 256:384]], axis=1),
        "wcd01": wcd0[:, 0:64],
        "wcd02": np.stack([wcd0[:, 64:128], wcd0[:, 128:192]], axis=1),
        "wcdy1": wcdy[:, 0:64],
        "wcdy2": np.stack([wcdy[:, 64:128], wcdy[:, 128:192]], axis=1),
        "wproj": Wp,
    }
    segs = []
    for nm, r, c, kind in _WLAYOUT:
        part = np.ascontiguousarray(wparts[nm])
        if kind == "bf16":
            assert part.shape == (r, c), (nm, part.shape)
            segs.append(part.astype(bf).ravel())
        else:
            assert part.shape == (r, 2, c), (nm, part.shape)
            raw = _q8(part).ravel().view(np.uint8)
            segs.append(raw.view(bf))
    blob = np.concatenate(segs)
    assert blob.shape == (WTOT,)

    shared = {"bias": bias}
    if not dist:
        shared["afull"] = a8t
        # reference fp8 B exactly as the device build would produce it
        A8m = an8.astype(f).reshape(N, N) / SA
        Bq = A8m @ A8m
        shared["bfull"] = _q8(Bq.T * SB).reshape(NT, P, N)
        shared["wfull"] = blob

    xs = np.asarray(inp["inputs"], f)[:, :, :, 0]    # (S, B, N)
    in_maps = []
    for b in range(NCORES):
        X = xs[:, b, :]                              # (S, N)
        AXt = X @ A.T                                # exact f32 (A@x).T rows
        BXt = AXt @ A.T
        xab = np.stack([X, AXt, BXt])                # (3, S, N)
        m = dict(shared)
        # rows (t, k): [x_t; (A@x_t); (B@x_t)] contiguous per step
        m["xab"] = np.ascontiguousarray(
            xab.transpose(1, 0, 2).reshape(3 * S, N)).astype(bf)
        m["aslc"] = np.ascontiguousarray(a8t[KSL * b:KSL * (b + 1)])
        if dist:
            m["wslc"] = np.ascontiguousarray(blob[WSL * b:WSL * (b + 1)])
        in_maps.append(m)
    return in_maps


_NC_CACHE = []


def kernel(**inputs):
    if not _NC_CACHE:
        _NC_CACHE.append(_build())
    nc = _NC_CACHE[0]
    in_maps = _host_prep(inputs)
    res = run_bass_kernel_spmd(nc, in_maps, core_ids=list(range(NCORES)))
    out = np.stack([res.results[b]["out"] for b in range(NCORES)], axis=1)
    return np.ascontiguousarray(out[..., None].astype(np.float32))


# revision 27
# speedup vs baseline: 1.4292x; 1.1731x over previous
"""Trainium2 Bass kernel for nn_CLCRNModel (CLCRN encoder-decoder GNN).

Strategy: data-parallel over batch (8 batch elements -> 8 NeuronCores).
The sparse 25-neighbor graph conv is cast as dense matmuls against the
row-normalized adjacency A and its square B = A^2, both SBUF-resident in
fp8-e4m3 and streamed through the PE with DoubleRow (2 fp8 MACs/cell).

Input-volume optimization (~12x fewer bytes per core than shipping the
dense operators): every core receives only its 2-k-tile slice of A^T
(512 KB fp8) and a 1/8 slice of a packed weight blob (fp8 dense-group
weights ride inside the bf16 blob as byte pairs, AP.bitcast on device).
On device: one HBM AllGather assembles the full A; each core transposes
its own slice on the PE (fp8 transpose, element-step-2 output) into the
natural-layout column block it needs to compute its 2 tile-rows of
B = A@A (fp8 DR); a second AllGather assembles the full B into the hop
operand. No dense matrix crosses the host-device link.

Per cell the PE runs two hop passes (gate: A@h,B@h; cand: A@rh,B@rh),
each 512-chunk accumulating both halves in PSUM with a shared stationary
(consecutive-Ldweights dedup), then scaled copies land [A@z|B@z] as an
fp8 DoubleRow-paired [64,2,N] dense operand. Dense gate/cand matmuls
are [h|node|x] (bf16, K=115) plus [A@z|B@z] (fp8 DR, K=128) groups.

Host-side linearity folds shrink every hop pass to the 64 hidden
channels: encoder feature-embedding and node embedding fold into
precomputed dense rows/biases (exact f32 A), and the decoder feedback
y_t = h_t @ W_proj + b_proj folds into the h-group dense weights.
"""
import os
import sys

for _p in ("/root/.axon_site/_ro/trn_rl_repo", "/opt/trn_rl_repo"):
    if os.path.isdir(_p) and _p not in sys.path:
        sys.path.append(_p)

import numpy as np
import ml_dtypes

import concourse.bass as bass
import concourse.mybir as mybir
import concourse.tile as tile
from concourse.bass_utils import run_bass_kernel_spmd
from concourse.masks import make_identity
from concourse.tile import add_dep_helper

P = 128
N = 2048
NT = 16            # node k-tiles
NPAIR = 8          # DoubleRow k-tile pairs
S = 12             # encoder steps
HOR = 12           # decoder steps
H = 64             # GRU units
FREE = 512         # hop chunk width (fp8 DR moving limit: 2x512)
NCH = N // FREE
NCORES = 8
KSL = NT // NCORES  # k-tiles per core slice
SA = 16.0          # fp8 scale for A
SB = 128.0         # fp8 scale for B (= SA^2 * 0.5, applied in B-build copy)

F32 = mybir.dt.float32
BF16 = mybir.dt.bfloat16
FP8 = mybir.dt.float8e4
AF = mybir.ActivationFunctionType
DR = mybir.MatmulPerfMode.DoubleRow

# packed weight blob (bf16-unit offsets). kind "bf16": [rows, cols] bf16
# tile; kind "fp8": [64, 2, cols] fp8 DoubleRow-paired tile, stored as raw
# byte pairs inside the bf16 blob (AP.bitcast on device).
# nodeT is [node; A@node; B@node].T
_WLAYOUT = [
    ("nodeT", 48, N, "bf16"),
    ("wge1", 115, 128, "bf16"), ("wge2", 64, 128, "fp8"),
    ("wce1", 115, 64, "bf16"), ("wce2", 64, 64, "fp8"),
    ("wgd01", 64, 128, "bf16"), ("wgd02", 64, 128, "fp8"),
    ("wgdf1", 64, 128, "bf16"), ("wgdf2", 64, 128, "fp8"),
    ("wcd01", 64, 64, "bf16"), ("wcd02", 64, 64, "fp8"),
    ("wcdy1", 64, 64, "bf16"), ("wcdy2", 64, 64, "fp8"),
    ("wproj", 64, 1, "bf16"),
]


def _wsize(r, c, kind):
    # bf16-unit count in the blob: fp8 parts are [r, 2, c] fp8 = r*c units
    return r * c if kind == "bf16" else r * c


WTOT = sum(_wsize(r, c, k) for _, r, c, k in _WLAYOUT)
assert WTOT % NCORES == 0
WSL = WTOT // NCORES


def _dedup_ldweights(nc):
    """Remove Ldweights whose weights AP equals the previous PE weight
    load (PE retains the stationary operand between matmuls; walrus's own
    ldw-opt is disabled in this toolchain). Waits/updates of a removed
    load migrate to the next PE instruction."""
    import concourse.mybir as _mb
    fn = nc.m.functions[0]
    pe = _mb.EngineType.PE
    n = 0
    for blk in fn.blocks:
        out = []
        last_sig = None
        pend_waits, pend_updates = [], []
        for ins in blk.instructions:
            if ins.engine == pe:
                if ins.opcode == "Ldweights":
                    sig = (str(ins.ins[0]), str(ins.tile_position),
                           str(ins.perf_mode), str(ins.is_transpose))
                    if sig == last_sig:
                        si = ins.sync_info
                        if si:
                            pend_waits.extend(si.on_wait or [])
                            pend_updates.extend(si.on_update or [])
                        n += 1
                        continue
                    last_sig = sig
                elif ins.opcode not in ("Matmult", "Drain", "EventSemaphore",
                                        "RegisterMove", "UnconditionalBranch"):
                    last_sig = None
                if pend_waits or pend_updates:
                    si = ins.sync_info
                    if si is None:
                        si = _mb.SyncInfo(on_wait=[], on_update=[])
                        ins.sync_info = si
                    si.on_wait = list(pend_waits) + list(si.on_wait or [])
                    si.on_update = list(si.on_update or []) + list(pend_updates)
                    pend_waits, pend_updates = [], []
            out.append(ins)
        assert not pend_waits and not pend_updates
        blk.instructions = out
    return n


def _split_multiwait(nc, max_waits=1):
    """This container's walrus rejects >1 sem-wait on CTRL-class
    instructions (the Tile exit drain carries one wait per live sem).
    Split excess waits onto preceding same-engine carrier drains."""
    fn = nc.m.functions[0]
    n = 0
    for blk in fn.blocks:
        out = []
        for ins in blk.instructions:
            si = ins.sync_info
            waits = list(si.on_wait) if (si and si.on_wait) else []
            if len(waits) > max_waits:
                extra, keep = waits[:-max_waits], waits[-max_waits:]
                for i in range(0, len(extra), max_waits):
                    carrier = mybir.InstDrain(
                        name=f"{ins.name}_wsplit{i}", ins=[], outs=[],
                        bass_is_fusable=False)
                    carrier.engine = ins.engine
                    carrier.sync_info = mybir.SyncInfo(
                        on_wait=extra[i:i + max_waits], on_update=[])
                    out.append(carrier)
                    n += 1
                si.on_wait = keep
            out.append(ins)
        blk.instructions = out
    return n


def _build(dist=True):
    nc = bass.Bass(num_devices=NCORES) if dist else bass.Bass()

    aslc_d = nc.dram_tensor("aslc", [KSL, P, N], FP8,
                            kind="ExternalInput")
    if dist:
        aslc_i = nc.dram_tensor("aslc_i", [KSL, P, N], FP8,
                                kind="Internal")
        a_g = nc.dram_tensor("a_g", [NT, P, N], FP8, kind="Internal",
                             addr_space="Shared")
        b_i = nc.dram_tensor("b_i", [KSL, P, N], FP8, kind="Internal")
        b_g = nc.dram_tensor("b_g", [NT, P, N], FP8, kind="Internal",
                             addr_space="Shared")
        wslc_d = nc.dram_tensor("wslc", [WSL], BF16, kind="ExternalInput")
        wslc_i = nc.dram_tensor("wslc_i", [WSL], BF16, kind="Internal")
        w_g = nc.dram_tensor("w_g", [WTOT], BF16, kind="Internal",
                             addr_space="Shared")
    else:
        afull_d = nc.dram_tensor("afull", [NT, P, N], FP8,
                                 kind="ExternalInput")
        bfull_d = nc.dram_tensor("bfull", [NT, P, N], FP8,
                                 kind="ExternalInput")
        wfull_d = nc.dram_tensor("wfull", [WTOT], BF16, kind="ExternalInput")
    xab_d = nc.dram_tensor("xab", [3 * S, N], BF16, kind="ExternalInput")
    bias_d = nc.dram_tensor("bias", [64, 12], F32, kind="ExternalInput")
    out_d = nc.dram_tensor("out", [HOR, N], BF16, kind="ExternalOutput")

    with tile.TileContext(nc) as tc:
        with tc.tile_pool(name="const", bufs=1) as cpool, \
             tc.tile_pool(name="state", bufs=1) as spool, \
             tc.tile_pool(name="psum", bufs=1, space="PSUM") as ppool:

            ab8 = cpool.tile([P, NT, 2 * N], FP8, name="ab8")
            ancol = cpool.tile([P, NT, KSL * P], FP8, name="ancol")
            aslcsb = cpool.tile([P, KSL, N], FP8, name="aslcsb")
            bstg = cpool.tile([P, KSL, N], FP8, name="bstg")
            wsb = {}
            for name, rows, cols, kind in _WLAYOUT:
                if name == "nodeT":
                    continue
                if kind == "bf16":
                    wsb[name] = cpool.tile([rows, cols], BF16, name=name)
                else:
                    wsb[name] = cpool.tile([rows, 2, cols], FP8, name=name)
            bias = cpool.tile([64, 12], F32, name="bias")
            identb = cpool.tile([P, P], BF16, name="identb")
            ident8 = cpool.tile([P, P], FP8, name="ident8")

            h_nat = spool.tile([P, NT, H], FP8, name="h_nat")
            rh_nat = spool.tile([P, NT, H], FP8, name="rh_nat")
            zxT = spool.tile([115, N], BF16, name="zxT")    # h | node | x
            abT = spool.tile([H, 2, N], FP8, name="abT")    # A@h | B@h
            rzxT = spool.tile([115, N], BF16, name="rzxT")  # rh | node | x
            rabT = spool.tile([H, 2, N], FP8, name="rabT")  # A@rh | B@rh
            xabsb = spool.tile([3 * S, N], BF16, name="xabsb")
            cT = spool.tile([H, N], BF16, name="cT")
            tmpT = spool.tile([H, N], BF16, name="tmpT")
            rT = spool.tile([H, N], BF16, name="rT")
            uT = spool.tile([H, N], BF16, name="uT")
            yT = spool.tile([1, N], BF16, name="yT")

            make_identity(nc, identb[:, :])
            nc.vector.tensor_copy(ident8[:, :], identb[:, :])

            # ---------- prologue: gathers + weight loads ----------
            cc_a = cc_w = None
            if dist:
                nc.sync.dma_start(aslc_i[:, :, :], aslc_d[:, :, :])
                cc_a = nc.gpsimd.collective_compute(
                    "AllGather", mybir.AluOpType.bypass,
                    replica_groups=[list(range(NCORES))],
                    ins=[aslc_i[:, :, :]], outs=[a_g[:, :, :]])
                nc.sync.dma_start(wslc_i[:], wslc_d[:])
                cc_w = nc.gpsimd.collective_compute(
                    "AllGather", mybir.AluOpType.bypass,
                    replica_groups=[list(range(NCORES))],
                    ins=[wslc_i[:]], outs=[w_g[:]])
                wsrc = w_g
            else:
                wsrc = wfull_d

            def gated_dma(dst, srcap, cc):
                # Tile tracks collective INPUT writers but not readers of
                # a collective's DRAM output; add the edge explicitly.
                d = nc.sync.dma_start(dst, srcap)
                if cc is not None:
                    add_dep_helper(d.ins, cc.ins, sync=True,
                                   reason="collective output read")
                return d

            off = 0
            for name, rows, cols, kind in _WLAYOUT:
                nun = _wsize(rows, cols, kind)
                if kind == "bf16":
                    srcap = wsrc[off:off + nun].rearrange(
                        "(a b) -> a b", a=rows, b=cols)
                else:
                    srcap = wsrc[off:off + nun].bitcast(FP8).rearrange(
                        "(a b c) -> a b c", a=rows, b=2, c=cols)
                if name == "nodeT":
                    gated_dma(zxT[64:112, :], srcap, cc_w)
                    gated_dma(rzxT[64:112, :], srcap, cc_w)
                elif kind == "bf16":
                    gated_dma(wsb[name][:, :], srcap, cc_w)
                else:
                    gated_dma(wsb[name][:, :, :], srcap, cc_w)
                off += nun
            nc.sync.dma_start(bias[:, :], bias_d[:, :])
            for tt in range(KSL):
                nc.sync.dma_start(aslcsb[:, tt, :], aslc_d[tt, :, :])
            asrc = a_g if dist else afull_d
            for k in range(NT):
                gated_dma(ab8[:, k, 0:N], asrc[k, :, :], cc_a)
            nc.sync.dma_start(xabsb[:, :], xab_d[:, :])
            nc.vector.memset(zxT[0:64, :], 0.0)
            nc.vector.memset(rzxT[0:64, :], 0.0)

            # ---------- helpers ----------
            def emit_ancol_build():
                # this core's natural-layout A column-slice = transpose of
                # its own T-slice k-tiles. fp8 PE transpose writes at
                # element step 2, so two blocks pack into one PSUM bank
                # and a single strided copy lands them in ancol.
                for j in range(NT):
                    tp8 = ppool.tile([P, FREE], FP8, name="tp8",
                                     tag="tp", bufs=2)
                    for tt in range(KSL):
                        nc.tensor.transpose(
                            tp8[:, tt * 2 * P:(tt + 1) * 2 * P:2],
                            aslcsb[:, tt, j * P:(j + 1) * P],
                            ident8[:, :])
                    if j % 2 == 0:
                        nc.vector.tensor_copy(ancol[:, j, :], tp8[:, ::2])
                    else:
                        nc.scalar.copy(ancol[:, j, :], tp8[:, ::2])

            def emit_b_build():
                # sharded B-build: this core computes only its KSL B^T
                # tile-rows (stationary = its own natural A column-slice,
                # moving = gathered T tiles), stages them to HBM, and an
                # AllGather assembles the full B in every core's ab8.
                # out_bp[p, f] = SA^2 * B^T[t*128+p, c*512+f]; stored
                # *SB/SA^2 as fp8.
                for tt in range(KSL):
                    bp = [ppool.tile([P, FREE], F32, name=f"bp{c}",
                                     tag="hp", bufs=4) for c in range(NCH)]
                    for l in range(NPAIR):
                        for c in range(NCH):
                            nc.tensor.matmul(
                                bp[c][:, :],
                                ancol[:, 2 * l:2 * l + 2,
                                      tt * P:(tt + 1) * P],
                                ab8[:, 2 * l:2 * l + 2,
                                    c * FREE:(c + 1) * FREE],
                                start=(l == 0), stop=(l == NPAIR - 1),
                                perf_mode=DR)
                    for c in range(NCH):
                        nc.vector.tensor_scalar_mul(
                            bstg[:, tt, c * FREE:(c + 1) * FREE],
                            bp[c][:, :], SB / (SA * SA))
                if dist:
                    for tt in range(KSL):
                        nc.sync.dma_start(b_i[tt, :, :], bstg[:, tt, :])
                    cc_b = nc.gpsimd.collective_compute(
                        "AllGather", mybir.AluOpType.bypass,
                        replica_groups=[list(range(NCORES))],
                        ins=[b_i[:, :, :]], outs=[b_g[:, :, :]])
                    for t in range(NT):
                        gated_dma(ab8[:, t, N:2 * N], b_g[t, :, :], cc_b)
                else:
                    # core 0's built tiles are t=0..KSL-1: use them so the
                    # sim verifies the on-device transpose+build path
                    for tt in range(KSL):
                        nc.sync.dma_start(ab8[:, tt, N:2 * N],
                                          bstg[:, tt, :])
                    for t in range(KSL, NT):
                        nc.sync.dma_start(ab8[:, t, N:2 * N],
                                          bfull_d[t, :, :])

            def hop_chunk(nat, dstT, c):
                # dstT chunk = [ (A@z).T ; (B@z).T ]. DR matmuls cannot
                # col-tile (ISA), so the two halves accumulate in separate
                # PSUM banks; each k-tile-pair stationary is shared by the
                # A/B matmul pair (consecutive Ldweights dedup).
                hpa = ppool.tile([P, FREE], F32, name="hpa", tag="hp",
                                 bufs=4)
                hpb = ppool.tile([P, FREE], F32, name="hpb", tag="hp",
                                 bufs=4)
                for jp in range(NPAIR):
                    st = nat[:, 2 * jp:2 * jp + 2, :]
                    nc.tensor.matmul(
                        hpa[0:H, :], st,
                        ab8[:, 2 * jp:2 * jp + 2,
                            c * FREE:(c + 1) * FREE],
                        start=(jp == 0), stop=(jp == NPAIR - 1),
                        perf_mode=DR)
                    nc.tensor.matmul(
                        hpb[0:H, :], st,
                        ab8[:, 2 * jp:2 * jp + 2,
                            N + c * FREE:N + (c + 1) * FREE],
                        start=(jp == 0), stop=(jp == NPAIR - 1),
                        perf_mode=DR)
                sl = slice(c * FREE, (c + 1) * FREE)
                # the PSUM halves carry the fp8 A/B scale factors; divide
                # them out here so the fp8 dense weights keep their
                # natural magnitude (W/SA in fp8 would be subnormal).
                # Engines alternate per chunk to balance DVE/Act load.
                if c % 2 == 0:
                    nc.vector.tensor_scalar_mul(dstT[0:H, 0, sl],
                                                hpa[0:H, :], 1.0 / SA)
                    nc.scalar.mul(dstT[0:H, 1, sl], hpb[0:H, :], 1.0 / SB)
                else:
                    nc.scalar.mul(dstT[0:H, 0, sl], hpa[0:H, :], 1.0 / SA)
                    nc.vector.tensor_scalar_mul(dstT[0:H, 1, sl],
                                                hpb[0:H, :], 1.0 / SB)

            def dense_chunk(groups, m, c):
                dp = ppool.tile([P, FREE], F32, name="dp", tag="dp",
                                bufs=2)
                ng = len(groups)
                for gi, (w_ap, rhs, kr, mode) in enumerate(groups):
                    if mode is None:
                        mov = rhs[0:kr, c * FREE:(c + 1) * FREE]
                    else:
                        mov = rhs[0:H, :, c * FREE:(c + 1) * FREE]
                    nc.tensor.matmul(
                        dp[0:m, :], w_ap, mov,
                        start=(gi == 0), stop=(gi == ng - 1),
                        perf_mode=mode)
                return dp

            def to_nat_group(srcT, dst, g):
                # natural fp8 tiles for 4 k-tiles: 4 transposes batched per
                # psum tile, one cast copy
                j0 = 4 * g
                tp = ppool.tile([P, 4 * H], BF16, name="tp", tag="tp",
                                bufs=2)
                for jj in range(4):
                    nc.tensor.transpose(
                        tp[:, jj * H:(jj + 1) * H],
                        srcT[0:H, (j0 + jj) * P:(j0 + jj + 1) * P],
                        identb[0:H, 0:H])
                if g % 2 == 1:
                    nc.vector.tensor_copy(dst[:, j0:j0 + 4, :], tp[:, :])
                else:
                    nc.scalar.copy(dst[:, j0:j0 + 4, :], tp[:, :])

            def make_rh_chunk(c):
                sl = slice(c * FREE, (c + 1) * FREE)
                nc.vector.tensor_mul(rzxT[0:H, sl], rT[:, sl],
                                     zxT[0:H, sl])
                to_nat_group(rzxT, rh_nat, c)

            def update_chunk(c, last):
                # h' = c + u*(h-c)
                sl = slice(c * FREE, (c + 1) * FREE)
                nc.vector.tensor_sub(tmpT[:, sl], zxT[0:H, sl], cT[:, sl])
                nc.vector.tensor_mul(tmpT[:, sl], tmpT[:, sl], uT[:, sl])
                nc.vector.tensor_add(zxT[0:H, sl], tmpT[:, sl], cT[:, sl])
                if not last:
                    to_nat_group(zxT, h_nat, c)

            def gate_chunk(groups, rcol, ucol, with_rh, c):
                dp = dense_chunk(groups, 128, c)
                sl = slice(c * FREE, (c + 1) * FREE)
                nc.scalar.activation(rT[:, sl], dp[0:64, :], AF.Sigmoid,
                                     bias=bias[:, rcol:rcol + 1])
                nc.scalar.activation(uT[:, sl], dp[64:128, :], AF.Sigmoid,
                                     bias=bias[:, ucol:ucol + 1])
                if with_rh:
                    make_rh_chunk(c)

            def cand_chunk(groups, bcol, last, c):
                dp = dense_chunk(groups, 64, c)
                sl = slice(c * FREE, (c + 1) * FREE)
                nc.scalar.activation(cT[:, sl], dp[0:64, :], AF.Tanh,
                                     bias=bias[:, bcol:bcol + 1])
                update_chunk(c, last)

            # ---------- encoder ----------
            def enc_step(t):
                nc.sync.dma_start(zxT[112:115, :], xabsb[3 * t:3 * t + 3, :])
                nc.sync.dma_start(rzxT[112:115, :],
                                  xabsb[3 * t:3 * t + 3, :])
                have_h = t > 0
                # t=0: h == 0 exactly -> A@h/B@h groups contribute +0.0,
                # and the h rows of zxT are memset; drop the ab groups.
                g_groups = [(wsb["wge1"][:, :], zxT, 115, None)]
                c_groups = [(wsb["wce1"][:, :], rzxT, 115, None)]
                if have_h:
                    g_groups.append((wsb["wge2"][:, :, :], abT, None, DR))
                    c_groups.append((wsb["wce2"][:, :, :], rabT, None, DR))
                for c in range(NCH):
                    if have_h:
                        hop_chunk(h_nat, abT, c)
                    if c > 0:
                        gate_chunk(g_groups, 0, 1, have_h, c - 1)
                gate_chunk(g_groups, 0, 1, have_h, NCH - 1)
                for c in range(NCH):
                    if have_h:
                        hop_chunk(rh_nat, rabT, c)
                    if c > 0:
                        cand_chunk(c_groups, 2, False, c - 1)
                cand_chunk(c_groups, 2, False, NCH - 1)

            # A-column + B builds are emitted before the (hop-free) t=0
            # cell: the B AllGather chain is the startup critical path,
            # while t=0's dense work overlaps the B exchange.
            emit_ancol_build()
            emit_b_build()
            enc_step(0)
            for t in range(1, S):
                enc_step(t)

            # ---------- decoder ----------
            for u in range(HOR):
                wg1 = wsb["wgd01"] if u == 0 else wsb["wgdf1"]
                wg2 = wsb["wgd02"] if u == 0 else wsb["wgdf2"]
                g_groups = [(wg1[:, :], zxT, 64, None),
                            (wg2[:, :, :], abT, None, DR)]
                c_groups = [(wsb["wcd01"][:, :], rzxT, 64, None),
                            (wsb["wcd02"][:, :, :], rabT, None, DR)]
                if u > 0:
                    c_groups = [(wsb["wcdy1"][:, :], zxT, 64, None),
                                (wsb["wcdy2"][:, :, :], abT, None, DR)] \
                        + c_groups
                rc, uc = (3, 4) if u == 0 else (5, 6)
                for c in range(NCH):
                    hop_chunk(h_nat, abT, c)
                    if c > 0:
                        gate_chunk(g_groups, rc, uc, True, c - 1)
                gate_chunk(g_groups, rc, uc, True, NCH - 1)
                bc = 7 if u == 0 else 8
                for c in range(NCH):
                    hop_chunk(rh_nat, rabT, c)
                    if c > 0:
                        cand_chunk(c_groups, bc, u == HOR - 1, c - 1)
                cand_chunk(c_groups, bc, u == HOR - 1, NCH - 1)
                # y = h' @ Wproj + b  (output only; feedback is folded)
                for c in range(NCH):
                    yp = ppool.tile([P, FREE], F32, name="yp", tag="dp",
                                    bufs=2)
                    nc.tensor.matmul(yp[0:1, :], wsb["wproj"][:, :],
                                     zxT[0:H, c * FREE:(c + 1) * FREE],
                                     start=True, stop=True)
                    nc.scalar.activation(yT[0:1, c * FREE:(c + 1) * FREE],
                                         yp[0:1, :], AF.Identity,
                                         bias=bias[0:1, 9:10])
                nc.sync.dma_start(out_d[u:u + 1, :], yT[:, :])

    _dedup_ldweights(nc)
    _split_multiwait(nc)
    return nc


# ---------------- host-side preprocessing ----------------

def _softplus(x):
    return np.log1p(np.exp(-np.abs(x))) + np.maximum(x, 0.0)


def _q8(x):
    # TRN e4m3 overflows to inf above +-240 (unlike OCP's 448): clip first.
    return np.clip(np.asarray(x, np.float32), -240.0, 240.0).astype(
        ml_dtypes.float8_e4m3)


def _host_prep(inp, dist=True):
    """Edge-weight MLP + row-normalization + fp8 A slices + all linearity
    folds. Pure per-graph preprocessing (no time loop). A-natural and
    B = A@A are built on-device; x/node multi-hop rows use exact f32 A."""
    f = np.float32
    bf = ml_dtypes.bfloat16
    row, col = np.asarray(inp["sparse_idx"])
    loc = np.asarray(inp["loc"], f)
    delta = loc[col] - loc[row]
    h1 = np.tanh(delta @ np.asarray(inp["Wk0"], f) + np.asarray(inp["bk0"], f))
    h2 = np.tanh(h1 @ np.asarray(inp["Wk1"], f) + np.asarray(inp["bk1"], f))
    ker = _softplus((h2 @ np.asarray(inp["Wk2"], f)
                     + np.asarray(inp["bk2"], f))[:, 0])
    geo = np.asarray(inp["geodesic"], f)
    w = ker * np.asarray(inp["angle_ratio"], f) * np.exp(-geo * geo)
    denom = np.zeros(N, f)
    np.add.at(denom, row, w)
    w = (w / (denom[row] + np.float32(1e-8))).astype(f)
    A = np.zeros((N, N), f)
    np.add.at(A, (row, col), w)

    # fp8 A: T-layout k-tiles (hop moving operand) and natural row-tiles
    # (B-build stationary): a8t[k, p, m] = A[m, k*128+p] * SA,
    # an8[j, p, i] = A[j*128+p, i] * SA
    a8t = _q8(A.T * SA).reshape(NT, P, N)
    an8 = _q8(A * SA).reshape(NT, P, N)

    Wfe = np.asarray(inp["W_fe"], f)      # (1, 16)
    bfe = np.asarray(inp["b_fe"], f)
    Wp = np.asarray(inp["W_proj"], f)     # (64, 1)
    bp = np.asarray(inp["b_proj"], f)
    node = np.asarray(inp["node_emb"], f)
    SC = [1.0, SA, SB]

    # encoder fold: z rows per hop k are [feat16 | node16 | x1 | h64].
    # x/node hop rows are computed with exact f32 A on the host (no SC).
    def enc_fold(W):
        out = W.shape[1]
        Wx = np.zeros((3, out), f)
        b_extra = np.zeros(out, f)
        Wh = np.zeros((64, 3 * out), f)
        for k in range(3):
            Wk = W[k * 97:(k + 1) * 97]
            Wf, Wxr, Whk = Wk[0:16], Wk[32:33], Wk[33:97]
            Wx[k] = Wxr[0] + Wfe[0] @ Wf
            b_extra += bfe @ Wf
            Wh[:, k * out:(k + 1) * out] = Whk
        return Wx, Wh, b_extra

    Wg_e = np.asarray(inp["Wg_e"], f)
    Wc_e = np.asarray(inp["Wc_e"], f)
    wgx, wge, bg_x = enc_fold(Wg_e)
    wcx, wce, bc_x = enc_fold(Wc_e)
    bg_e = np.asarray(inp["bg_e"], f) + bg_x
    bc_e = np.asarray(inp["bc_e"], f) + bc_x

    # node rhs rows: [node.T; (A node).T; (B node).T] exact f32; per-hop
    # weight blocks stacked in wgn/wcn rows 0-47; rows 48-50 hold the
    # folded x/Ax/Bx weights (rhs rows DMA'd per step)
    Anode = A @ node
    nodeT = np.concatenate([node.T, Anode.T, (A @ Anode).T], axis=0)
    wgn = np.zeros((51, 128), f)
    wcn = np.zeros((51, 64), f)
    for k in range(3):
        wgn[k * 16:(k + 1) * 16] = Wg_e[k * 97 + 16:k * 97 + 32]
        wcn[k * 16:(k + 1) * 16] = Wc_e[k * 97 + 16:k * 97 + 32]
    wgn[48:51] = wgx
    wcn[48:51] = wcx

    # decoder fold: z rows per hop k are [y1 | h64]
    Wg_d = np.asarray(inp["Wg_d"], f)
    Wc_d = np.asarray(inp["Wc_d"], f)

    def dec_fold(W):
        out = W.shape[1]
        Wh_plain = np.zeros((64, 3 * out), f)
        Wh_fold = np.zeros((64, 3 * out), f)
        Wy_h = np.zeros((64, 3 * out), f)
        b_extra = np.zeros(out, f)
        for k in range(3):
            Wk = W[k * 65:(k + 1) * 65]
            Wy, Wh = Wk[0:1], Wk[1:65]
            Wh_plain[:, k * out:(k + 1) * out] = Wh
            Wh_fold[:, k * out:(k + 1) * out] = Wh + Wp @ Wy
            Wy_h[:, k * out:(k + 1) * out] = Wp @ Wy
            b_extra += bp @ Wy
        return Wh_plain, Wh_fold, Wy_h, b_extra

    wgd0, wgdf, _, bgd_x = dec_fold(Wg_d)
    wcd0, _, wcdy, bcd_x = dec_fold(Wc_d)
    bg_d = np.asarray(inp["bg_d"], f)
    bc_d = np.asarray(inp["bc_d"], f)

    bias = np.zeros((64, 12), f)
    bias[:, 0] = bg_e[0:64]
    bias[:, 1] = bg_e[64:128]
    bias[:, 2] = bc_e
    bias[:, 3] = bg_d[0:64]
    bias[:, 4] = bg_d[64:128]
    bias[:, 5] = (bg_d + bgd_x)[0:64]
    bias[:, 6] = (bg_d + bgd_x)[64:128]
    bias[:, 7] = bc_d
    bias[:, 8] = bc_d + bcd_x
    bias[0, 9] = bp[0]

    # pack the dense-weight blob in _WLAYOUT order:
    # group 1 rhs is [h | node | x] (K=115), group 2 rhs is [A@z|B@z]
    wparts = {
        "nodeT": nodeT,
        "wge1": np.concatenate([wge[:, 0:128], wgn]),
        "wge2": np.stack([wge[:, 128:256], wge[:, 256:384]], axis=1),
        "wce1": np.concatenate([wce[:, 0:64], wcn]),
        "wce2": np.stack([wce[:, 64:128], wce[:, 128:192]], axis=1),
        "wgd01": wgd0[:, 0:128],
        "wgd02": np.stack([wgd0[:, 128:256], wgd0[:, 256:384]], axis=1),
        "wgdf1": wgdf[:, 0:128],
        "wgdf2": np.stack([wgdf[:, 128:256], wgdf[:, 256:384]], axis=1),
        "wcd01": wcd0[:, 0:64],
        "wcd02": np.stack([wcd0[:, 64:128], wcd0[:, 128:192]], axis=1),
        "wcdy1": wcdy[:, 0:64],
        "wcdy2": np.stack([wcdy[:, 64:128], wcdy[:, 128:192]], axis=1),
        "wproj": Wp,
    }
    segs = []
    for nm, r, c, kind in _WLAYOUT:
        part = np.ascontiguousarray(wparts[nm])
        if kind == "bf16":
            assert part.shape == (r, c), (nm, part.shape)
            segs.append(part.astype(bf).ravel())
        else:
            assert part.shape == (r, 2, c), (nm, part.shape)
            raw = _q8(part).ravel().view(np.uint8)
            segs.append(raw.view(bf))
    blob = np.concatenate(segs)
    assert blob.shape == (WTOT,)

    shared = {"bias": bias}
    if not dist:
        shared["afull"] = a8t
        # reference fp8 B exactly as the device build would produce it
        A8m = an8.astype(f).reshape(N, N) / SA
        Bq = A8m @ A8m
        shared["bfull"] = _q8(Bq.T * SB).reshape(NT, P, N)
        shared["wfull"] = blob

    xs = np.asarray(inp["inputs"], f)[:, :, :, 0]    # (S, B, N)
    in_maps = []
    for b in range(NCORES):
        X = xs[:, b, :]                              # (S, N)
        AXt = X @ A.T                                # exact f32 (A@x).T rows
        BXt = AXt @ A.T
        xab = np.stack([X, AXt, BXt])                # (3, S, N)
        m = dict(shared)
        # rows (t, k): [x_t; (A@x_t); (B@x_t)] contiguous per step
        m["xab"] = np.ascontiguousarray(
            xab.transpose(1, 0, 2).reshape(3 * S, N)).astype(bf)
        m["aslc"] = np.ascontiguousarray(a8t[KSL * b:KSL * (b + 1)])
        if dist:
            m["wslc"] = np.ascontiguousarray(blob[WSL * b:WSL * (b + 1)])
        in_maps.append(m)
    return in_maps


_NC_CACHE = []


def kernel(**inputs):
    if not _NC_CACHE:
        _NC_CACHE.append(_build())
    nc = _NC_CACHE[0]
    in_maps = _host_prep(inputs)
    res = run_bass_kernel_spmd(nc, in_maps, core_ids=list(range(NCORES)))
    out = np.stack([res.results[b]["out"] for b in range(NCORES)], axis=1)
    return np.ascontiguousarray(out[..., None].astype(np.float32))
